# revision 10
# baseline (speedup 1.0000x reference)
"""Trainium2 Bass kernel v7: cross-entropy with Gaussian-smoothed labels.

loss = mean over tokens of  [ Wsum(t) * logsumexp(pred_row) - sum_k w_k * pred[start+k] ]

Device-side:
  - stream pred [8192, 722] f32 per core through SBUF in [128, G*722]
    tiles; per-token sum(exp) load-balanced between ACT (exp+HW
    accumulator) and DVE (batched exp + reduce_sum); one Ln -> lse.
  - the 7-tap windowed term is fetched with 4 dma_gather ops (one per
    2048-row range; int16 block indices, 64-f32 blocks at 64-element
    stride) and contracted against a host-built weight tensor with
    affine_mul_reduce -> a single [P,1] partial. 64-aligned blocks mean
    ~9% of windows straddle two blocks; those tokens get two gather
    slots with the weights split across them.
Host-side (_shard_inputs): pure target preprocessing - block indices,
    the [slots, 64] tap-placement weights (bf16), per-token weight sums.
"""
import math

import numpy as np

import concourse.bass as bass
import concourse.bacc as bacc
import concourse.tile as tile
from concourse import mybir
from concourse import bass_utils

B, T, C = 32, 2048, 722
CORES = 8
SHARD = B * T // CORES          # 8192 tokens per core
P = 128
TILES = SHARD // P              # 64
K = 7
START_MAX = C - K               # 715
DECAYS = [math.exp(-(2.0 ** d) / 4.0) for d in range(4)]

OPS = 4                         # dma_gather ops per core
ROWS_PER_OP = SHARD // OPS      # 2048
SLOTS = 2432                    # 19*128 slots per op (2048 + straddle margin)
SCOLS = SLOTS // 128            # 19
ICOLS = SLOTS // 16             # idx columns (idx i at [i%16, i//16])
NBLK = (ROWS_PER_OP * C - 64) // 64 + 1

_ALU = mybir.AluOpType
_ACT = mybir.ActivationFunctionType

_F = np.zeros(13, np.float32)
for _m in range(4):
    _F[6 + _m] = _F[6 - _m] = DECAYS[_m]
_F[6] = 1.0    # exact target position overwrites decay[0]

_NC = None


def _build(G=4, acc_set=(3, 6, 9, 12, 15), pred_bufs=4, exp_bufs=3):
    ngroups = TILES // G
    acc_set = set(acc_set)
    nc = bacc.Bacc("TRN2", target_bir_lowering=False, debug=False,
                   enable_asserts=True, num_devices=CORES)
    pred = nc.dram_tensor("pred", [SHARD, C], mybir.dt.float32, kind="ExternalInput")
    gidx = nc.dram_tensor("gidx", [OPS * 128, ICOLS], mybir.dt.int16,
                          kind="ExternalInput")
    gw = nc.dram_tensor("gw", [OPS * SLOTS, 64], mybir.dt.bfloat16,
                        kind="ExternalInput")
    wsum_in = nc.dram_tensor("wsum", [SHARD], mybir.dt.float32,
                             kind="ExternalInput")
    out = nc.dram_tensor("partial", [P, 1], mybir.dt.float32, kind="ExternalOutput")

    # token index = p*TILES + jg*G + g  (each partition owns a contiguous slab)
    pred_g = pred.ap().rearrange("(p j g) c -> j p g c", p=P, g=G)

    with tile.TileContext(nc) as tc:
        with (tc.tile_pool(name="pred", bufs=pred_bufs) as pred_pool,
              tc.tile_pool(name="exp", bufs=exp_bufs) as exp_pool,
              tc.tile_pool(name="small", bufs=1) as small):
            # gather-side constants via the scalar ring (sync ring stays
            # exclusive to the pred stream)
            idx_sb = small.tile([128, OPS, ICOLS], mybir.dt.int16)
            nc.scalar.dma_start(out=idx_sb,
                                in_=gidx.ap().rearrange("(b p) s -> p b s", p=128))
            gw_sb = small.tile([128, OPS, SCOLS, 64], mybir.dt.bfloat16)
            nc.scalar.dma_start(
                out=gw_sb,
                in_=gw.ap().rearrange("(b c p) k -> p b c k", p=128, c=SCOLS))
            wsum_sb = small.tile([P, TILES], mybir.dt.float32)
            nc.scalar.dma_start(out=wsum_sb,
                                in_=wsum_in.ap().rearrange("(p j) -> p j", p=P))

            # ordering fence: a Pool-engine op reading idx_sb, so the
            # gathers behind it on the Q7 queue cannot race the idx DMA
            idx_fence = small.tile([128, OPS, ICOLS], mybir.dt.int16)
            nc.gpsimd.tensor_copy(out=idx_fence, in_=idx_sb)

            blk = small.tile([128, OPS, SCOLS, 64], mybir.dt.float32)
            for b in range(OPS):
                src = bass.AP(tensor=pred, offset=b * ROWS_PER_OP * C,
                              ap=[[64, NBLK], [1, 64]])
                nc.gpsimd.dma_gather(
                    out_ap=blk[:, b], in_ap=src, idxs_ap=idx_sb[:, b],
                    num_idxs=SLOTS, num_idxs_reg=SLOTS,
                    elem_size=64, elem_step=64, single_packet=False)

            # dense stream: per-token sum(exp), split ACT-accum / DVE-reduce.
            # Every ACTIVATE carries accum_out (the HW accumulator register
            # is stateful across ops; pairing each op with a read is the
            # proven-correct pattern); batched groups dump theirs into a
            # junk column.
            sums = small.tile([P, TILES], mybir.dt.float32)
            junk = small.tile([P, 1], mybir.dt.float32)
            for jg in range(ngroups):
                pt = pred_pool.tile([P, G, C], mybir.dt.float32)
                nc.sync.dma_start(out=pt, in_=pred_g[jg])
                if jg in acc_set:
                    for g in range(G):
                        j = jg * G + g
                        et = exp_pool.tile([P, C], mybir.dt.float32, tag="acc")
                        nc.scalar.activation(out=et, in_=pt[:, g, :], func=_ACT.Exp,
                                             accum_out=sums[:, j:j + 1])
                else:
                    et = exp_pool.tile([P, G, C], mybir.dt.float32, tag="dve")
                    nc.scalar.activation(out=et, in_=pt, func=_ACT.Exp,
                                         accum_out=junk)
                    nc.vector.reduce_sum(out=sums[:, jg * G:(jg + 1) * G], in_=et,
                                         axis=mybir.AxisListType.X)

            # gather-side contraction (after the stream in program order —
            # DVE is in-order and must not head-of-line block the reduces).
            # bf16 weights -> f32 once, then 4 chained mul-reduces.
            gwf = small.tile([128, OPS, SCOLS, 64], mybir.dt.float32)
            nc.vector.tensor_copy(out=gwf, in_=gw_sb)
            wgd = small.tile([128, SCOLS, 64], mybir.dt.float32)
            gparts = small.tile([P, OPS], mybir.dt.float32)
            for b in range(OPS):
                nc.vector.affine_mul_reduce(
                    out=wgd, accum_out=gparts[:, b:b + 1],
                    in0=blk[:, b], in1=gwf[:, b], scale=1.0, bias=0.0)

            # lse and the final partial
            lse = small.tile([P, TILES], mybir.dt.float32)
            nc.scalar.activation(out=lse, in_=sums, func=_ACT.Ln)
            loss = small.tile([P, TILES], mybir.dt.float32)
            nc.vector.tensor_mul(out=loss, in0=wsum_sb, in1=lse)
            spart = small.tile([P, 1], mybir.dt.float32)
            nc.vector.reduce_sum(out=spart, in_=loss, axis=mybir.AxisListType.X)
            gpart = small.tile([P, 1], mybir.dt.float32)
            nc.vector.reduce_sum(out=gpart, in_=gparts, axis=mybir.AxisListType.X)
            part = small.tile([P, 1], mybir.dt.float32)
            nc.vector.tensor_sub(out=part, in0=spart, in1=gpart)
            nc.sync.dma_start(out=out.ap(), in_=part)
    nc.compile()
    return nc


def _get_nc():
    global _NC
    if _NC is None:
        _NC = _build()
    return _NC


def _gather_plan(target_shard):
    """Host-side target preprocessing: block indices, tap-placement
    weights, per-token weight sums for one core's shard."""
    tgt = target_shard.astype(np.int64)
    s = np.clip(tgt - 3, 0, START_MAX)
    u = (tgt - s).astype(np.int64)
    t = np.arange(SHARD, dtype=np.int64)
    O = t * C + s
    wk = _F[6 + (np.arange(K)[None, :] - u[:, None])]          # [SHARD, 7]
    op = t // ROWS_PER_OP
    rel = O - op * ROWS_PER_OP * C
    blk = rel // 64
    q = rel % 64
    straddle = q > 64 - K

    idx_arr = np.zeros((OPS, 128, ICOLS), np.int16)
    w_arr = np.zeros((OPS, SLOTS, 64), np.float32)
    ar_k = np.arange(K)[None, :]
    for b in range(OPS):
        sel = np.nonzero(op == b)[0]
        sec = sel[straddle[sel]]
        n0, n1 = sel.size, sec.size
        assert n0 + n1 <= SLOTS, (n0, n1)
        eidx = np.zeros(SLOTS, np.int64)
        eidx[:n0] = blk[sel]
        eidx[n0:n0 + n1] = blk[sec] + 1
        cols = q[sel][:, None] + ar_k                           # [n0, 7]
        valid = cols < 64
        rows = np.broadcast_to(np.arange(n0)[:, None], cols.shape)
        w_arr[b][rows[valid], cols[valid]] = wk[sel][valid]
        if n1:
            cols2 = q[sec][:, None] + ar_k - 64
            valid2 = cols2 >= 0
            rows2 = np.broadcast_to(np.arange(n0, n0 + n1)[:, None], cols2.shape)
            w_arr[b][rows2[valid2], cols2[valid2]] = wk[sec][valid2]
        idx_arr[b] = np.tile(eidx.astype(np.int16).reshape(SLOTS // 16, 16).T,
                             (8, 1))
    import ml_dtypes
    return (idx_arr.reshape(OPS * 128, ICOLS),
            w_arr.reshape(OPS * SLOTS, 64).astype(ml_dtypes.bfloat16),
            wk.sum(axis=1).astype(np.float32))


def _shard_inputs(pred, target):
    bpc = B // CORES
    in_maps = []
    for c in range(CORES):
        tgt_shard = np.ascontiguousarray(
            target[c * bpc:(c + 1) * bpc].reshape(SHARD), dtype=np.int32)
        gidx, gw, wsum = _gather_plan(tgt_shard)
        in_maps.append({
            "pred": np.ascontiguousarray(
                pred[c * bpc:(c + 1) * bpc].reshape(SHARD, C), dtype=np.float32),
            "gidx": gidx,
            "gw": gw,
            "wsum": wsum,
        })
    return in_maps


def _run(pred, target, **kwargs):
    nc = _get_nc()
    return bass_utils.run_bass_kernel_spmd(
        nc, _shard_inputs(pred, target), core_ids=list(range(CORES)), **kwargs)


def kernel(pred, target):
    res = _run(pred, target)
    total = sum(float(r["partial"].astype(np.float64).sum()) for r in res.results)
    return np.asarray(total / (B * T), dtype=np.float32)


# revision 11
# speedup vs baseline: 1.4280x; 1.4280x over previous
"""Trainium2 Bass kernel v8: cross-entropy with Gaussian-smoothed labels.

loss = mean over tokens of  [ Wsum(t) * logsumexp(pred_row) - sum_k w_k * pred[start+k] ]

Device-side, per core (8-way batch-parallel, 8192 tokens each):
  - stream pred [8192, 722] f32 through SBUF in [128, G*722] tiles;
    per-token sum(exp) is load-balanced between ACT (exp + HW
    accumulator, one op per token column) and DVE (batched exp +
    reduce_sum per group) so neither engine outruns the DMA stream;
    one Ln at the end -> lse.
  - the 7-tap windowed term: 64 indirect DMAs (one offset per
    partition per op - the only offset layout the SWDGE firmware
    honors; Q7 descriptor emission runs at ~8 ns/descriptor, so these
    ~71 us overlap the whole stream). Window offsets, tap weights and
    per-token weight sums are host-shipped (pure target preprocessing).
  - everything after the stream is wrapped in tile_wait_until so the
    Tile scheduler cannot hoist gather-consumers into the in-order DVE
    queue ahead of the stream reduces (its SWDGE cost model is ~10x
    optimistic, which otherwise head-of-line blocks the stream).
  - per-core partial sums [128, 1] DMA'd out; host sums 8x128 and divides.
"""
import math

import numpy as np

import concourse.bass as bass
import concourse.bacc as bacc
import concourse.tile as tile
from concourse import mybir
from concourse import bass_utils

B, T, C = 32, 2048, 722
CORES = 8
SHARD = B * T // CORES          # 8192 tokens per core
P = 128
TILES = SHARD // P              # 64
K = 7
START_MAX = C - K               # 715
DECAYS = [math.exp(-(2.0 ** d) / 4.0) for d in range(4)]

_ALU = mybir.AluOpType
_ACT = mybir.ActivationFunctionType

_F = np.zeros(13, np.float32)
for _m in range(4):
    _F[6 + _m] = _F[6 - _m] = DECAYS[_m]
_F[6] = 1.0    # exact target position overwrites decay[0]

_NC = None


def _build(G=4, acc_set=(3, 6, 9, 12, 15), pred_bufs=4, exp_bufs=3,
           tail_wait_ms=1.0):
    ngroups = TILES // G
    acc_set = set(acc_set)
    nc = bacc.Bacc("TRN2", target_bir_lowering=False, debug=False,
                   enable_asserts=True, num_devices=CORES)
    pred = nc.dram_tensor("pred", [SHARD, C], mybir.dt.float32, kind="ExternalInput")
    goffs = nc.dram_tensor("goffs", [SHARD], mybir.dt.int32, kind="ExternalInput")
    gwk = nc.dram_tensor("gwk", [SHARD * K], mybir.dt.float32, kind="ExternalInput")
    wsum_in = nc.dram_tensor("wsum", [SHARD], mybir.dt.float32, kind="ExternalInput")
    out = nc.dram_tensor("partial", [P, 1], mybir.dt.float32, kind="ExternalOutput")

    pred_flat = pred.ap().rearrange("a b -> (a b)").rearrange("(n one) -> n one", one=1)
    # token index = p*TILES + jg*G + g  (each partition owns a contiguous slab)
    pred_g = pred.ap().rearrange("(p j g) c -> j p g c", p=P, g=G)

    with tile.TileContext(nc) as tc:
        with (tc.tile_pool(name="pred", bufs=pred_bufs) as pred_pool,
              tc.tile_pool(name="exp", bufs=exp_bufs) as exp_pool,
              tc.tile_pool(name="small", bufs=1) as small):
            # host-precomputed gather constants via the scalar ring (sync
            # ring stays exclusive to the pred stream)
            offs = small.tile([P, TILES], mybir.dt.int32)
            nc.scalar.dma_start(out=offs,
                                in_=goffs.ap().rearrange("(p j) -> p j", p=P))
            wk_sb = small.tile([P, TILES, K], mybir.dt.float32)
            nc.scalar.dma_start(
                out=wk_sb,
                in_=gwk.ap().rearrange("(p j k) -> p j k", p=P, k=K))
            wsum_sb = small.tile([P, TILES], mybir.dt.float32)
            nc.scalar.dma_start(out=wsum_sb,
                                in_=wsum_in.ap().rearrange("(p j) -> p j", p=P))

            # ordering fence: a Pool-engine op reading offs so the gathers
            # behind it on the Q7 queue cannot race the offs DMA
            offs_fence = small.tile([P, TILES], mybir.dt.int32)
            nc.gpsimd.tensor_copy(out=offs_fence, in_=offs)

            # windowed gathers: one indirect DMA per token-tile, one offset
            # per partition (the only HW-correct layout)
            gath = small.tile([P, TILES, K], mybir.dt.float32)
            for j in range(TILES):
                nc.gpsimd.indirect_dma_start(
                    out=gath[:, j, :],
                    out_offset=None,
                    in_=pred_flat,
                    in_offset=bass.IndirectOffsetOnAxis(ap=offs[:, j:j + 1], axis=0),
                )

            # dense stream: per-token sum(exp), split ACT-accum / DVE-reduce.
            # Every ACTIVATE carries accum_out (the HW accumulator register
            # is stateful across ops; pairing each op with a read is the
            # proven-correct pattern); batched groups dump theirs into a
            # junk column.
            sums = small.tile([P, TILES], mybir.dt.float32)
            junk = small.tile([P, 1], mybir.dt.float32)
            for jg in range(ngroups):
                pt = pred_pool.tile([P, G, C], mybir.dt.float32)
                nc.sync.dma_start(out=pt, in_=pred_g[jg])
                if jg in acc_set:
                    for g in range(G):
                        j = jg * G + g
                        et = exp_pool.tile([P, C], mybir.dt.float32, tag="acc")
                        nc.scalar.activation(out=et, in_=pt[:, g, :], func=_ACT.Exp,
                                             accum_out=sums[:, j:j + 1])
                else:
                    et = exp_pool.tile([P, G, C], mybir.dt.float32, tag="dve")
                    nc.scalar.activation(out=et, in_=pt, func=_ACT.Exp,
                                         accum_out=junk)
                    nc.vector.reduce_sum(out=sums[:, jg * G:(jg + 1) * G], in_=et,
                                         axis=mybir.AxisListType.X)

            # tail: force-scheduled after the stream (and after the gathers
            # finish for the DVE side) so nothing head-of-line blocks the
            # in-order engine queues mid-stream.
            with tc.tile_wait_until(tail_wait_ms):
                lse = small.tile([P, TILES], mybir.dt.float32)
                nc.scalar.activation(out=lse, in_=sums, func=_ACT.Ln)
                wg = small.tile([P, TILES, K], mybir.dt.float32)
                gsum = small.tile([P, TILES], mybir.dt.float32)
                nc.vector.tensor_mul(out=wg, in0=wk_sb, in1=gath)
                nc.vector.reduce_sum(out=gsum, in_=wg, axis=mybir.AxisListType.X)
                loss = small.tile([P, TILES], mybir.dt.float32)
                nc.vector.tensor_mul(out=loss, in0=wsum_sb, in1=lse)
                nc.vector.tensor_sub(out=loss, in0=loss, in1=gsum)
                part = small.tile([P, 1], mybir.dt.float32)
                nc.vector.reduce_sum(out=part, in_=loss, axis=mybir.AxisListType.X)
                nc.sync.dma_start(out=out.ap(), in_=part)
    nc.compile()
    return nc


def _get_nc():
    global _NC
    if _NC is None:
        _NC = _build()
    return _NC


def _gather_plan(target_shard):
    """Host-side target preprocessing: flat window-start offsets, tap
    weights [SHARD, 7], per-token weight sums."""
    tgt = target_shard.astype(np.int64)
    s = np.clip(tgt - 3, 0, START_MAX)
    u = (tgt - s).astype(np.int64)
    t = np.arange(SHARD, dtype=np.int64)
    offs = (t * C + s).astype(np.int32)
    wk = _F[6 + (np.arange(K)[None, :] - u[:, None])].astype(np.float32)
    wsum = wk.sum(axis=1).astype(np.float32)
    # device layouts: token t at (p, j) = (t // TILES, t % TILES)
    tok = np.arange(SHARD).reshape(P, TILES)
    return (offs[tok].reshape(SHARD),
            wk[tok].reshape(SHARD * K),
            wsum[tok].reshape(SHARD))


def _shard_inputs(pred, target):
    bpc = B // CORES
    in_maps = []
    for c in range(CORES):
        tgt_shard = np.ascontiguousarray(
            target[c * bpc:(c + 1) * bpc].reshape(SHARD), dtype=np.int32)
        goffs, gwk, wsum = _gather_plan(tgt_shard)
        in_maps.append({
            "pred": np.ascontiguousarray(
                pred[c * bpc:(c + 1) * bpc].reshape(SHARD, C), dtype=np.float32),
            "goffs": goffs,
            "gwk": gwk,
            "wsum": wsum,
        })
    return in_maps


def _run(pred, target, **kwargs):
    nc = _get_nc()
    return bass_utils.run_bass_kernel_spmd(
        nc, _shard_inputs(pred, target), core_ids=list(range(CORES)), **kwargs)


def kernel(pred, target):
    res = _run(pred, target)
    total = sum(float(r["partial"].astype(np.float64).sum()) for r in res.results)
    return np.asarray(total / (B * T), dtype=np.float32)
